# revision 1
# baseline (speedup 1.0000x reference)
"""LoRA linear kernel for 8 TRN2 NeuronCores — fp16 host-cast.

out = x @ (base_weight + SCALE * lora_B @ lora_A).T + bias
for x [4, 2048, 4096], base_weight [4096, 4096], rank 8.

Sharding ('r2c4'): 2 token-halves x 4 d_out-quarters = 8 cores
(tensor-parallel on d_out per the hint, plus a token split that keeps
per-core x traffic low and W' SBUF-resident at 32 KB/partition).

Host prep: W' = W + SCALE*B@A is computed in numpy (268 MFLOP) and both
x and W'.T are cast to fp16 and pre-tiled, so HBM traffic halves vs f32
(32 MB x + 8 MB W' + 16 MB f32 out per core) and there is no device-side
LoRA prep phase. fp16 operand rounding gives ~3e-4 rel L2 error.

Per core: W' lives in SBUF as 32 [128, 1024] fp16 k-tiles. For each of
32 128-token tiles: one 1 MB contiguous DMA loads the pre-tiled x.T
block; 32x2 accumulating [128k,128t]x[128k,512o] matmuls (x k-tile
stationary, W' moving, fp16 at 1 col/cycle) fill 2 PSUM banks; the DVE
adds bias during PSUM->SBUF copyback; out rows DMA back contiguously.
x and out buffers are deep (see nb) and PSUM rotates 4 tile-sets across
all 8 banks, which lets the Tile scheduler weave a 4-wide token-tile
interleave with no PE gaps.

Measured floor analysis (same-process A/B, slope over an on-device
For_i repeat loop, 8 cores concurrent): walrus emits LDWEIGHTS before
EVERY matmul (no elision; --enable-ldw-opt crashes; moving free dim is
ISA-capped at 512), and the ~53 ns weight load serializes with the
213 ns stream, so the practical floor is 2048 x ~265 ns ~= 543 us/pass
per core against the 437 us pure-streaming roofline. Probed and
rejected: 64x128 row-tiled K-split ping-pong (equal), 128x64 column
tiling (slower, 2x instructions), 3D-AP matmul splitting (ISA check
rejects), fp8 DoubleRow (~3-5% error vs the 2e-2 gate). This kernel
measures 531-550 us/pass depending on device state, vs 592 us for the
f32r baseline.
"""
import sys

if '/opt/trn_rl_repo' not in sys.path:
    sys.path.insert(0, '/opt/trn_rl_repo')

from contextlib import ExitStack

import numpy as np

import concourse.bacc as bacc
import concourse.mybir as mybir
import concourse.tile as tile
from concourse.bass_utils import run_bass_kernel_spmd

SCALE = 16.0 / 8.0  # alpha / rank

P = 128
K = 4096           # d_in (contraction)
KT = K // P        # 32 k-tiles
D_OUT = 4096
B, S = 4, 2048
T_FULL = B * S     # 8192 tokens
N_CORES = 8

MODE = 'r2c4'
# mode: (r_split, c_split, mm_width)
MODES = {'r2c4': (2, 4, 512), 'r4c2': (4, 2, 512),
         'r2c4w': (2, 4, 1024), 'r4c2w': (4, 2, 1024)}

_nc_cache = {}


def _dims(mode):
    r_split, c_split, mmw = MODES[mode]
    t_core = T_FULL // r_split
    tt = t_core // P
    o_core = D_OUT // c_split
    oc = o_core // mmw
    return r_split, c_split, t_core, tt, o_core, oc, mmw


def build_nc(repeat=1, mode=None):
    mode = mode or MODE
    key = (mode, repeat)
    if key in _nc_cache:
        return _nc_cache[key]
    f32 = mybir.dt.float32
    f16 = mybir.dt.float16
    _, _, t_core, TT, O_CORE, OC, MMW = _dims(mode)

    nc = bacc.Bacc(None, target_bir_lowering=False)
    # x blocks: [t_tile, p(k-within-tile), kt, j(token-within-tile)]
    xb = nc.dram_tensor("xb", [TT, P, KT, P], f16, kind="ExternalInput")
    wt = nc.dram_tensor("wt", [KT, P, O_CORE], f16, kind="ExternalInput")
    biasb = nc.dram_tensor("biasb", [P, O_CORE], f32, kind="ExternalInput")
    out = nc.dram_tensor("out", [t_core, O_CORE], f32, kind="ExternalOutput")

    with ExitStack() as ctx:
        tc = ctx.enter_context(tile.TileContext(nc))
        wpool = ctx.enter_context(tc.tile_pool(name="wpool", bufs=1))
        cpool = ctx.enter_context(tc.tile_pool(name="cpool", bufs=1))
        # PSUM: 8 banks of 512 f32; each psum tile spans MMW/512 banks.
        banks_per_tile = OC * (MMW // 512)
        psbufs = max(2, 8 // banks_per_tile)
        nb = 2 if O_CORE == 2048 else 6
        xpool = ctx.enter_context(tc.tile_pool(name="xpool", bufs=nb))
        opool = ctx.enter_context(tc.tile_pool(name="opool", bufs=nb))
        pspool = ctx.enter_context(tc.tile_pool(name="ps", bufs=psbufs,
                                                space="PSUM"))

        bias_t = cpool.tile([P, O_CORE], f32, tag="bias")
        nc.sync.dma_start(bias_t[:], biasb[:])

        # ---- W' tiles resident in SBUF as [k, o], fp16 ----
        wtiles = []
        for k in range(KT):
            w_t = wpool.tile([P, O_CORE], f16, tag=f"w{k}")
            nc.sync.dma_start(w_t[:], wt[k])
            wtiles.append(w_t)

        # ---- main loop: out[t, o] = x_tile.T @ W' (+ bias) ----
        def load_x(tt):
            xt = xpool.tile([P, KT, P], f16, name=f"xt_{tt}", tag="xt")
            nc.sync.dma_start(xt[:], xb[tt])
            return xt

        def alloc_ps(tt):
            return [pspool.tile([P, MMW], f32, tag=f"ps{oc}",
                                name=f"ps_{tt}_{oc}")
                    for oc in range(OC)]

        def flush(tt, pss):
            o_t = opool.tile([P, O_CORE], f32, name=f"ot_{tt}", tag="ot")
            for oc in range(OC):
                sl = slice(oc * MMW, (oc + 1) * MMW)
                nc.vector.tensor_add(o_t[:, sl], pss[oc][:], bias_t[:, sl])
            nc.sync.dma_start(out[tt * P:(tt + 1) * P, :], o_t[:])

        def mms(xt, pss, k):
            for oc in range(OC):
                nc.tensor.matmul(
                    pss[oc][:],
                    xt[:, k, :],
                    wtiles[k][:, oc * MMW:(oc + 1) * MMW],
                    start=(k == 0), stop=(k == KT - 1),
                )

        # First INTRO token tiles interleaved k-major: the PE consumes each
        # W' k-tile INTRO*OC times as it streams in from HBM on the first
        # pass. INTRO tiles' PSUM banks stay live (psbufs rotation).
        INTRO = max(1, min(TT, psbufs))

        def main_pass():
            ixt = [load_x(tt) for tt in range(INTRO)]
            ips = [alloc_ps(tt) for tt in range(INTRO)]
            for k in range(KT):
                for tt in range(INTRO):
                    mms(ixt[tt], ips[tt], k)
            for tt in range(INTRO):
                flush(tt, ips[tt])
            for tt in range(INTRO, TT):
                xt = load_x(tt)
                pss = alloc_ps(tt)
                for k in range(KT):
                    mms(xt, pss, k)
                flush(tt, pss)

        if repeat == 1:
            main_pass()
        else:
            with tc.For_i(0, repeat, 1):
                main_pass()

    nc.compile()
    _nc_cache[key] = nc
    return nc


def _prep_in_maps(x, base_weight, lora_A, lora_B, bias, mode=None):
    mode = mode or MODE
    r_split, c_split, t_core, TT, O_CORE, OC, MMW = _dims(mode)
    w_full = base_weight.astype(np.float32) + \
        SCALE * (lora_B.astype(np.float32) @ lora_A.astype(np.float32))
    WT = np.ascontiguousarray(w_full.T).astype(np.float16)  # [k, o]
    x2d = x.reshape(T_FULL, K).astype(np.float16)
    bias = bias.astype(np.float32, copy=False)

    xbs = []
    for h in range(r_split):
        xh = x2d[h * t_core:(h + 1) * t_core]
        # [tt, j(tok), kt, p(k)] -> [tt, p, kt, j]
        xb = np.ascontiguousarray(
            xh.reshape(TT, P, KT, P).transpose(0, 3, 2, 1))
        xbs.append(xb)

    in_maps = []
    for h in range(r_split):
        for q in range(c_split):
            osl = slice(q * O_CORE, (q + 1) * O_CORE)
            wtq = np.ascontiguousarray(
                WT[:, osl].reshape(KT, P, O_CORE))
            biasb = np.ascontiguousarray(
                np.broadcast_to(bias[osl][None, :], (P, O_CORE)))
            in_maps.append({"xb": xbs[h], "wt": wtq, "biasb": biasb})
    return in_maps


def _assemble(results, mode=None):
    mode = mode or MODE
    r_split, c_split, t_core, TT, O_CORE, OC, MMW = _dims(mode)
    flat = np.empty((T_FULL, D_OUT), dtype=np.float32)
    i = 0
    for h in range(r_split):
        for q in range(c_split):
            flat[h * t_core:(h + 1) * t_core,
                 q * O_CORE:(q + 1) * O_CORE] = results[i]["out"]
            i += 1
    return flat.reshape(B, S, D_OUT)


def kernel(x, base_weight, lora_A, lora_B, bias):
    x = np.asarray(x)
    base_weight = np.asarray(base_weight)
    lora_A = np.asarray(lora_A)
    lora_B = np.asarray(lora_B)
    bias = np.asarray(bias)
    nc = build_nc()
    in_maps = _prep_in_maps(x, base_weight, lora_A, lora_B, bias)
    res = run_bass_kernel_spmd(nc, in_maps, core_ids=list(range(N_CORES)))
    return _assemble(res.results)

